# revision 30
# baseline (speedup 1.0000x reference)
"""Causal multi-head attention layer on 8 trn2 NeuronCores.

Sharding: 8 cores = 4 batches x 2 head-groups. Core c handles batch c//2 and
heads [8*(c%2), 8*(c%2)+8). Each core runs QKV projections for its 512-wide
head slice, causal flash attention for 8 heads, and a partial output
projection (its 512 rows of Wo). Host sums the two partials per batch + bo.

Problem constants (hardcoded per contract): B=4, L=2048, D=1024, H=16, DK=DV=64.
"""

import sys

import os
for _p in ("/opt/trn_rl_repo", "/root/.axon_site/_ro/trn_rl_repo"):
    if os.path.isdir(_p) and _p not in sys.path:
        sys.path.insert(0, _p)

import numpy as np
import ml_dtypes

import concourse.bass as bass
import concourse.tile as tile
from concourse import bacc, mybir
from concourse.bass_utils import run_bass_kernel_spmd
BF16 = ml_dtypes.bfloat16

B, L, D, H, DK, DV = 4, 2048, 1024, 16, 64, 64
N_CORES = 8
HL = 8          # heads per core
DH = 512        # local head dim (HL * DK)
P = 128
LC = 512        # l-chunk
NLC = L // LC   # 4
NDC = D // P    # 8 contraction chunks for projections
NKC = DH // P   # 4 dk chunks
NST = L // P    # 16 s tiles
SCALE = 1.0 / np.sqrt(DK)

TRACE = False          # set by test harness for profiling runs
LAST_RESULTS = None    # BassKernelResults of the last run (for profiling)

_COMPILED = None


def _build():
    f32 = mybir.dt.float32
    bf16 = mybir.dt.bfloat16
    AF = mybir.ActivationFunctionType

    nc = bacc.Bacc("TRN2", target_bir_lowering=False, debug=False,
                   num_devices=N_CORES)

    xqT = nc.dram_tensor("xqT", [NLC, P, NDC, LC], bf16, kind="ExternalInput").ap()
    xkT = nc.dram_tensor("xkT", [NLC, P, NDC, LC], bf16, kind="ExternalInput").ap()
    xvT = nc.dram_tensor("xvT", [NLC, P, NDC, LC], bf16, kind="ExternalInput").ap()
    # wq/wk are kc-major so per-kc weight slices are contiguous DMAs
    wq = nc.dram_tensor("wq", [NKC, P, NDC, P], bf16, kind="ExternalInput").ap()
    wk = nc.dram_tensor("wk", [NKC, P, NDC, P], bf16, kind="ExternalInput").ap()
    wv = nc.dram_tensor("wv", [P, NDC, DH], bf16, kind="ExternalInput").ap()
    wo = nc.dram_tensor("wo", [P, NKC, D], bf16, kind="ExternalInput").ap()
    bq = nc.dram_tensor("bq", [P, NKC], f32, kind="ExternalInput").ap()
    bk = nc.dram_tensor("bk", [P, NKC], f32, kind="ExternalInput").ap()
    bv = nc.dram_tensor("bv", [P, DH], f32, kind="ExternalInput").ap()
    outp = nc.dram_tensor("outp", [L, D], f32, kind="ExternalOutput").ap()

    from contextlib import ExitStack

    with tile.TileContext(nc) as tc, ExitStack() as ctx:
        const = ctx.enter_context(tc.tile_pool(name="const", bufs=1))
        kvp = ctx.enter_context(tc.tile_pool(name="kv", bufs=1))
        xp = ctx.enter_context(tc.tile_pool(name="x", bufs=2))
        qp = ctx.enter_context(tc.tile_pool(name="qt", bufs=2))
        ptp = ctx.enter_context(tc.tile_pool(name="pt", bufs=6))
        atp = ctx.enter_context(tc.tile_pool(name="at", bufs=2))
        osb = ctx.enter_context(tc.tile_pool(name="osb", bufs=3))
        nrm = ctx.enter_context(tc.tile_pool(name="nrm", bufs=3))
        ps_pj = ctx.enter_context(tc.tile_pool(name="ps_pj", bufs=2, space="PSUM"))
        ps_s = ctx.enter_context(tc.tile_pool(name="ps_s", bufs=2, space="PSUM"))
        ps_o = ctx.enter_context(tc.tile_pool(name="ps_o", bufs=2, space="PSUM"))

        # ---- initial loads: 3 DMA queues, need-ordered. Startup is
        # HBM-bound (~6MB before attention(0)), so critical bytes go first.
        wq_sb = const.tile([P, NKC, NDC, P], bf16, tag="wq")
        wk_sb = const.tile([P, NKC, NDC, P], bf16, tag="wk")
        wv_sb = const.tile([P, NDC, DH], bf16, tag="wv")
        x0 = [xp.tile([P, NDC, LC], bf16, tag=nm, name=nm)
              for nm in ("xq", "xk", "xv")]
        # sync: wq kc0 -> xq dc0-3 in quarters -> wq rest -> xk quarters
        nc.sync.dma_start(wq_sb[:, 0], wq[0])
        nc.sync.dma_start(x0[0][:, 0:2], xqT[0][:, 0:2])
        nc.sync.dma_start(x0[0][:, 2:4], xqT[0][:, 2:4])
        for kc in range(1, NKC):
            nc.sync.dma_start(wq_sb[:, kc], wq[kc])
        nc.sync.dma_start(x0[1][:, 0:2], xkT[0][:, 0:2])
        nc.sync.dma_start(x0[1][:, 2:4], xkT[0][:, 2:4])
        nc.sync.dma_start(x0[2][:], xvT[0])
        # scalar: xq dc4-7 -> wk 0,1 -> xk rest -> biases -> wo
        nc.scalar.dma_start(x0[0][:, 4:6], xqT[0][:, 4:6])
        nc.scalar.dma_start(x0[0][:, 6:8], xqT[0][:, 6:8])
        nc.scalar.dma_start(wk_sb[:, 0], wk[0])
        nc.scalar.dma_start(wk_sb[:, 1], wk[1])
        nc.scalar.dma_start(x0[1][:, 4:6], xkT[0][:, 4:6])
        nc.scalar.dma_start(x0[1][:, 6:8], xkT[0][:, 6:8])
        bq_sb = const.tile([P, NKC], f32, tag="bq")
        nc.scalar.dma_start(bq_sb[:], bq[:])
        bk_sb = const.tile([P, NKC], f32, tag="bk")
        nc.scalar.dma_start(bk_sb[:], bk[:])
        bv_sb = const.tile([P, DH], f32, tag="bv")
        nc.scalar.dma_start(bv_sb[:], bv[:])
        wo_sb = const.tile([P, NKC, D], bf16, tag="wo")
        # gpsimd: wk 2,3 -> wv (wo is deferred until the startup crunch ends)
        nc.gpsimd.dma_start(wk_sb[:, 2], wk[2])
        nc.gpsimd.dma_start(wk_sb[:, 3], wk[3])
        nc.gpsimd.dma_start(wv_sb[:], wv[:])

        # ---- PE warmup: batches of matmuls with no data deps, interleaved
        # between the first projection groups. They soak up DMA-wait gaps so
        # HAM reaches (and keeps) K=8/8 through the byte-bound start phase.
        warm = const.tile([P, LC], bf16, tag="warm")
        nc.vector.memset(warm[:], 1.0)

        def warmup(n):
            wps = ps_pj.tile([P, LC], f32, tag="ps_pj", name="warm")
            for _ in range(n):
                nc.tensor.matmul(wps[0:DV, :], warm[:, 0:DV], warm[:],
                                 start=True, stop=True, skip_group_check=True)
            # reader so the verifier accepts the warmup psum writes and the
            # pool ring advances
            nc.vector.tensor_copy(warm[0:1, 0:1], wps[0:1, 0:1])

        # persistent K^T and V. Each (st, head) slab is [ones(64) | V(64)]:
        # the leading ones replicate the softmax denominator onto psum
        # partitions 0-63 (O^T lands on 64-127), so normalization needs no
        # partition moves. The big ones memset runs on gpsimd, off the
        # DVE/critical path, while startup DMAs are still in flight.
        kT_sb = kvp.tile([P, NKC, L], bf16, tag="kT")
        vall = kvp.tile([P, NST * HL * P], bf16, tag="v")
        vv_c = vall[:].rearrange("p (n c) -> p n c", c=P)
        nc.gpsimd.memset(
            vall[:].rearrange("p (n t c) -> p n t c", t=2, c=DV)[:, :, 0, :], 1.0)

        W = {"q": (wq_sb, bq_sb), "k": (wk_sb, bk_sb)}

        def load_x(lc):
            xs = []
            for nm, dram, q in (("xq", xqT, nc.scalar), ("xk", xkT, nc.gpsimd),
                                ("xv", xvT, nc.gpsimd)):
                t = xp.tile([P, NDC, LC], bf16, tag=nm, name=nm)
                q.dma_start(t[:], dram[lc])
                xs.append(t)
            return xs

        def evict_qk(which, kc, ps, qt_t, lc):
            b_sb = W[which][1]
            dst = (qt_t[:, kc, :] if which == "q"
                   else kT_sb[:, kc, bass.ts(lc, LC)])
            nc.vector.tensor_scalar_add(dst, ps[:], b_sb[:, kc:kc + 1])

        def evict_v(st, ps):
            vw = (vall[:].rearrange("p (n t c) -> p n t c", t=2, c=DV)
                  [:, st * HL:(st + 1) * HL, 1, :])
            nc.vector.tensor_tensor(
                vw, ps[:].rearrange("p (h c) -> p h c", c=DV),
                bv_sb[:].rearrange("p (h c) -> p h c", c=DV),
                mybir.AluOpType.add)

        def proj_groups(lc, xs, qt_t):
            """Filler granularity: one closure per 8-matmul group."""
            xq_t, xk_t, xv_t = xs
            groups = []

            def qk_group(which, kc):
                def emit():
                    w_sb = W[which][0]
                    x_t = xq_t if which == "q" else xk_t
                    ps = ps_pj.tile([P, LC], f32, tag="ps_pj", name="ps_pj")
                    for dc in range(NDC):
                        nc.tensor.matmul(ps[:], w_sb[:, kc, dc, :],
                                         x_t[:, dc, :],
                                         start=(dc == 0), stop=(dc == NDC - 1))
                    evict_qk(which, kc, ps, qt_t, lc)
                return emit

            def v_group(j):
                def emit():
                    st = lc * (LC // P) + j
                    ps = ps_pj.tile([P, LC], f32, tag="ps_pj", name="ps_pj")
                    for dc in range(NDC):
                        nc.tensor.matmul(ps[:], xv_t[:, dc, bass.ts(j, P)],
                                         wv_sb[:, dc, :],
                                         start=(dc == 0), stop=(dc == NDC - 1))
                    evict_v(st, ps)
                return emit

            for kc in range(NKC):
                groups.append(qk_group("q", kc))
            for kc in range(NKC):
                groups.append(qk_group("k", kc))
            for j in range(LC // P):
                groups.append(v_group(j))
            return groups

        def outproj_groups(lc, at_t):
            groups = []

            def op_group(lt):
                def emit():
                    o_sb = osb.tile([P, D], f32, tag="o_sb", name="o_sb")
                    for n in range(2):
                        ps = ps_pj.tile([P, LC], f32, tag="ps_pj", name="ps_pj")
                        for hc in range(NKC):
                            nc.tensor.matmul(ps[:], at_t[hc][:, bass.ts(lt, P)],
                                             wo_sb[:, hc, bass.ts(n, 512)],
                                             start=(hc == 0), stop=(hc == NKC - 1))
                        nc.vector.tensor_copy(o_sb[:, bass.ts(n, 512)], ps[:])
                    nc.sync.dma_start(
                        outp[lc * LC + lt * P: lc * LC + (lt + 1) * P, :], o_sb[:])
                return emit

            for lt in range(LC // P):
                groups.append(op_group(lt))
            return groups

        def attention(lc, qt_t, fillers, pops=(2, 2, 2, 2)):
            n_st = (lc + 1) * (LC // P)
            at_t = [atp.tile([P, LC], bf16, tag=f"at{hc}", name=f"at{hc}")
                    for hc in range(NKC)]
            for hp in range(NKC):
                h0, h1 = 2 * hp, 2 * hp + 1
                po0 = ps_o.tile([P, LC], f32, tag="ps_o", name="ps_o")
                po1 = ps_o.tile([P, LC], f32, tag="ps_o", name="ps_o")

                def mm1(st):
                    # S^T: two heads packed on PE row halves, one 2-bank psum
                    jj = st - lc * (LC // P)
                    nc0 = jj * P if jj >= 0 else 0
                    s01 = ps_s.tile([P, 2 * LC], f32, tag="ps_s", name="ps_s")
                    nc.tensor.matmul(s01[:, nc0:LC], kT_sb[0:64, hp, bass.ts(st, P)],
                                     qt_t[0:64, hp, nc0:], start=True, stop=True,
                                     tile_position=(0, 0))
                    nc.tensor.matmul(s01[:, LC + nc0:], kT_sb[64:128, hp, bass.ts(st, P)],
                                     qt_t[64:128, hp, nc0:], start=True, stop=True,
                                     tile_position=(64, 0))
                    return s01, nc0

                pend = mm1(0)
                for st in range(n_st):
                    s01, nc0 = pend
                    if st + 1 < n_st:
                        pend = mm1(st + 1)  # PE one step ahead of ACT
                    pt01 = ptp.tile([P, 2 * LC], bf16, tag="pt", name="pt")
                    nc.scalar.activation(
                        pt01[:].rearrange("p (t c) -> p t c", t=2)[:, :, nc0:],
                        s01[:].rearrange("p (t c) -> p t c", t=2)[:, :, nc0:],
                        AF.Exp, bias=0.0, scale=float(SCALE))
                    if st >= lc * (LC // P):
                        # causal mask: zero exp() on the diagonal 128x128
                        # block where l < s (gpsimd, off the DVE/ACT path)
                        dview = (pt01[:].rearrange("p (t c) -> p t c", t=2)
                                 [:, :, nc0:nc0 + P])
                        nc.gpsimd.affine_select(
                            out=dview, in_=dview,
                            compare_op=mybir.AluOpType.is_ge,
                            fill=0.0, base=0,
                            pattern=[[0, 2], [1, P]], channel_multiplier=-1,
                        )
                    nc.tensor.matmul(po0[:, nc0:], vv_c[:, st * HL + h0, :],
                                     pt01[:, nc0:LC],
                                     start=(st == 0), stop=(st == n_st - 1))
                    nc.tensor.matmul(po1[:, nc0:], vv_c[:, st * HL + h1, :],
                                     pt01[:, LC + nc0:],
                                     start=(st == 0), stop=(st == n_st - 1))

                # PE filler(s) emitted first: they are independent of the
                # normalize chain and keep PE fed while it runs
                for _ in range(pops[hp]):
                    if fillers:
                        fillers.pop(0)()

                # normalize straight out of PSUM: den replicated on
                # partitions 0-63, O^T on 64-127
                rb0 = nrm.tile([64, LC], f32, tag="rb", name="rb")
                nc.vector.reciprocal_approx_fast(rb0[:], po0[0:64, :])
                nc.vector.tensor_mul(at_t[hp][0:64, :], po0[64:128, :], rb0[:])
                rb1 = nrm.tile([64, LC], f32, tag="rb", name="rb")
                nc.vector.reciprocal_approx_fast(rb1[:], po1[0:64, :])
                nc.vector.tensor_mul(at_t[hp][64:128, :], po1[64:128, :], rb1[:])
            return at_t

        # ---- pipeline ----
        # proj(0) runs alone, with warmup batches between groups to soak up
        # DMA-wait gaps; proj(lc+1)/outproj(lc-1) groups fill
        # normalize-chain gaps in attention(lc). proj(3) is split: its
        # kc=0,1 q/k groups + v groups run inside attention(2); its kc=2,3
        # q/k groups run inside attention(3) just before the head pairs
        # that need them.
        qt0 = qp.tile([P, NKC, LC], bf16, tag="qt", name="qt")
        pg0 = proj_groups(0, x0, qt0)
        warmup(16)
        for gi, g in enumerate(pg0):
            g()
            if gi < 10:
                warmup(4)
        nc.scalar.dma_start(wo_sb[:], wo[:])
        qt_cur = qt0
        at_prev = None
        pj3_late = []
        for lc in range(NLC):
            fillers = []
            if lc + 1 < NLC:
                xs = load_x(lc + 1)
                qt_nxt = qp.tile([P, NKC, LC], bf16, tag="qt", name="qt")
                pg = proj_groups(lc + 1, xs, qt_nxt)
                if lc + 1 == NLC - 1:
                    # q0,k0,q1,k1 + all v groups now; q2,k2,q3,k3 deferred
                    fillers += [pg[0], pg[4], pg[1], pg[5]] + pg[8:12]
                    pj3_late = [pg[2], pg[6], pg[3], pg[7]]
                else:
                    fillers += pg
            else:
                fillers += pj3_late
            if at_prev is not None:
                fillers += outproj_groups(lc - 1, at_prev)
            at_t = attention(lc, qt_cur, fillers,
                             pops=(2, 2, 2, 1) if lc == NLC - 1 else (2, 2, 2, 2))
            if lc == NLC - 1:
                # independent PE work covering the last pair's normalize
                # chain so HAM stays warm into the final out-projection
                warmup(8)
            for g in fillers:
                g()
            fillers.clear()
            if lc + 1 < NLC:
                qt_cur = qt_nxt
            at_prev = at_t
        for g in outproj_groups(NLC - 1, at_prev):
            g()

    nc.compile()
    return nc


def _get_compiled():
    global _COMPILED
    if _COMPILED is None:
        _COMPILED = _build()
    return _COMPILED


def kernel(queries, keys, values, Wq, bq, Wk, bk, Wv, bv, Wo, bo):
    global LAST_RESULTS
    nc = _get_compiled()

    queries = np.asarray(queries, np.float32)
    keys = np.asarray(keys, np.float32)
    values = np.asarray(values, np.float32)

    def pack_x(x):
        # (L, D) -> (NLC, P, NDC, LC): [lc, p, dc, l] = x[lc*LC+l, dc*P+p]
        t = x.T.reshape(NDC, P, NLC, LC)          # [dc, p, lc, l]
        return np.ascontiguousarray(t.transpose(2, 1, 0, 3)).astype(BF16)

    xT = {}
    for b in range(B):
        xT[("q", b)] = pack_x(np.asarray(queries[b]))
        xT[("k", b)] = pack_x(np.asarray(keys[b]))
        xT[("v", b)] = pack_x(np.asarray(values[b]))

    wslice = {}
    for g in range(2):
        sl = slice(DH * g, DH * (g + 1))

        def pack_w_kc(w):
            # (D, DH-slice) -> (NKC, P, NDC, P) kc-major
            return np.ascontiguousarray(
                w.reshape(NDC, P, NKC, P).transpose(2, 1, 0, 3)).astype(BF16)

        wslice[("wq", g)] = pack_w_kc(np.asarray(Wq, np.float32)[:, sl])
        wslice[("wk", g)] = pack_w_kc(np.asarray(Wk, np.float32)[:, sl])
        wslice[("wv", g)] = np.ascontiguousarray(
            np.asarray(Wv, np.float32)[:, sl]
            .reshape(NDC, P, DH).transpose(1, 0, 2)).astype(BF16)
        wslice[("wo", g)] = np.ascontiguousarray(
            np.asarray(Wo, np.float32)[sl, :].reshape(NKC, P, D).transpose(1, 0, 2)
        ).astype(BF16)
        wslice[("bq", g)] = np.ascontiguousarray(
            np.asarray(bq, np.float32)[sl].reshape(NKC, P).T)
        wslice[("bk", g)] = np.ascontiguousarray(
            np.asarray(bk, np.float32)[sl].reshape(NKC, P).T)
        wslice[("bv", g)] = np.ascontiguousarray(
            np.broadcast_to(np.asarray(bv, np.float32)[sl], (P, DH)))

    in_maps = []
    for c in range(N_CORES):
        b, g = c // 2, c % 2
        in_maps.append({
            "xqT": xT[("q", b)], "xkT": xT[("k", b)], "xvT": xT[("v", b)],
            "wq": wslice[("wq", g)], "wk": wslice[("wk", g)],
            "wv": wslice[("wv", g)], "wo": wslice[("wo", g)],
            "bq": wslice[("bq", g)], "bk": wslice[("bk", g)],
            "bv": wslice[("bv", g)],
        })

    res = run_bass_kernel_spmd(nc, in_maps, list(range(N_CORES)), trace=TRACE)
    LAST_RESULTS = res

    bo32 = np.asarray(bo, np.float32)
    out = np.empty((B, L, D), np.float32)
    for b in range(B):
        out[b] = res.results[2 * b]["outp"] + res.results[2 * b + 1]["outp"] + bo32
    return out


# revision 32
# speedup vs baseline: 1.0143x; 1.0143x over previous
"""Causal multi-head attention layer on 8 trn2 NeuronCores.

Sharding: 8 cores = 4 batches x 2 head-groups. Core c handles batch c//2 and
heads [8*(c%2), 8*(c%2)+8). Each core runs QKV projections for its 512-wide
head slice, causal flash attention for 8 heads, and a partial output
projection (its 512 rows of Wo). Host sums the two partials per batch + bo.

Problem constants (hardcoded per contract): B=4, L=2048, D=1024, H=16, DK=DV=64.
"""

import sys

import os
for _p in ("/opt/trn_rl_repo", "/root/.axon_site/_ro/trn_rl_repo"):
    if os.path.isdir(_p) and _p not in sys.path:
        sys.path.insert(0, _p)

import numpy as np
import ml_dtypes

import concourse.bass as bass
import concourse.tile as tile
from concourse import bacc, mybir
from concourse.bass_utils import run_bass_kernel_spmd
BF16 = ml_dtypes.bfloat16

B, L, D, H, DK, DV = 4, 2048, 1024, 16, 64, 64
N_CORES = 8
HL = 8          # heads per core
DH = 512        # local head dim (HL * DK)
P = 128
LC = 512        # l-chunk
NLC = L // LC   # 4
NDC = D // P    # 8 contraction chunks for projections
NKC = DH // P   # 4 dk chunks
NST = L // P    # 16 s tiles
SCALE = 1.0 / np.sqrt(DK)

TRACE = False          # set by test harness for profiling runs
LAST_RESULTS = None    # BassKernelResults of the last run (for profiling)

_COMPILED = None


def _build():
    f32 = mybir.dt.float32
    bf16 = mybir.dt.bfloat16
    AF = mybir.ActivationFunctionType

    nc = bacc.Bacc("TRN2", target_bir_lowering=False, debug=False,
                   num_devices=N_CORES)

    xqT = nc.dram_tensor("xqT", [NLC, P, NDC, LC], bf16, kind="ExternalInput").ap()
    xkT = nc.dram_tensor("xkT", [NLC, P, NDC, LC], bf16, kind="ExternalInput").ap()
    xvT = nc.dram_tensor("xvT", [NLC, P, NDC, LC], bf16, kind="ExternalInput").ap()
    # wq/wk are kc-major so per-kc weight slices are contiguous DMAs
    wq = nc.dram_tensor("wq", [NKC, P, NDC, P], bf16, kind="ExternalInput").ap()
    wk = nc.dram_tensor("wk", [NKC, P, NDC, P], bf16, kind="ExternalInput").ap()
    wv = nc.dram_tensor("wv", [P, NDC, DH], bf16, kind="ExternalInput").ap()
    wo = nc.dram_tensor("wo", [P, NKC, D], bf16, kind="ExternalInput").ap()
    bq = nc.dram_tensor("bq", [P, NKC], f32, kind="ExternalInput").ap()
    bk = nc.dram_tensor("bk", [P, NKC], f32, kind="ExternalInput").ap()
    bv = nc.dram_tensor("bv", [P, DH], f32, kind="ExternalInput").ap()
    outp = nc.dram_tensor("outp", [L, D], f32, kind="ExternalOutput").ap()

    from contextlib import ExitStack

    with tile.TileContext(nc) as tc, ExitStack() as ctx:
        const = ctx.enter_context(tc.tile_pool(name="const", bufs=1))
        kvp = ctx.enter_context(tc.tile_pool(name="kv", bufs=1))
        xp = ctx.enter_context(tc.tile_pool(name="x", bufs=2))
        qp = ctx.enter_context(tc.tile_pool(name="qt", bufs=2))
        ptp = ctx.enter_context(tc.tile_pool(name="pt", bufs=6))
        atp = ctx.enter_context(tc.tile_pool(name="at", bufs=2))
        osb = ctx.enter_context(tc.tile_pool(name="osb", bufs=3))
        nrm = ctx.enter_context(tc.tile_pool(name="nrm", bufs=3))
        ps_pj = ctx.enter_context(tc.tile_pool(name="ps_pj", bufs=2, space="PSUM"))
        ps_s = ctx.enter_context(tc.tile_pool(name="ps_s", bufs=2, space="PSUM"))
        ps_o = ctx.enter_context(tc.tile_pool(name="ps_o", bufs=2, space="PSUM"))

        # ---- initial loads: 3 DMA queues, need-ordered. Startup is
        # HBM-bound (~6MB before attention(0)), so critical bytes go first.
        wq_sb = const.tile([P, NKC, NDC, P], bf16, tag="wq")
        wk_sb = const.tile([P, NKC, NDC, P], bf16, tag="wk")
        wv_sb = const.tile([P, NDC, DH], bf16, tag="wv")
        x0 = [xp.tile([P, NDC, LC], bf16, tag=nm, name=nm)
              for nm in ("xq", "xk", "xv")]
        # sync: wq kc0 -> xq half -> wq rest -> xk half -> wv
        nc.sync.dma_start(wq_sb[:, 0], wq[0])
        nc.sync.dma_start(x0[0][:, 0:4], xqT[0][:, 0:4])
        for kc in range(1, NKC):
            nc.sync.dma_start(wq_sb[:, kc], wq[kc])
        nc.sync.dma_start(x0[1][:, 0:4], xkT[0][:, 0:4])
        nc.sync.dma_start(wv_sb[:], wv[:])
        # scalar: xq other half -> wk 0,1 -> xk other half -> biases
        nc.scalar.dma_start(x0[0][:, 4:8], xqT[0][:, 4:8])
        nc.scalar.dma_start(wk_sb[:, 0], wk[0])
        nc.scalar.dma_start(wk_sb[:, 1], wk[1])
        nc.scalar.dma_start(x0[1][:, 4:8], xkT[0][:, 4:8])
        bq_sb = const.tile([P, NKC], f32, tag="bq")
        nc.scalar.dma_start(bq_sb[:], bq[:])
        bk_sb = const.tile([P, NKC], f32, tag="bk")
        nc.scalar.dma_start(bk_sb[:], bk[:])
        bv_sb = const.tile([P, DH], f32, tag="bv")
        nc.scalar.dma_start(bv_sb[:], bv[:])
        wo_sb = const.tile([P, NKC, D], bf16, tag="wo")
        # gpsimd: wk 2,3 -> xv (wo is deferred until the startup crunch ends)
        nc.gpsimd.dma_start(wk_sb[:, 2], wk[2])
        nc.gpsimd.dma_start(wk_sb[:, 3], wk[3])
        nc.gpsimd.dma_start(x0[2][:], xvT[0])

        # ---- PE warmup: batches of matmuls with no data deps, interleaved
        # between the first projection groups. They soak up DMA-wait gaps so
        # HAM reaches (and keeps) K=8/8 through the byte-bound start phase.
        warm = const.tile([P, LC], bf16, tag="warm")
        nc.vector.memset(warm[:], 1.0)

        def warmup(n):
            wps = ps_pj.tile([P, LC], f32, tag="ps_pj", name="warm")
            for _ in range(n):
                nc.tensor.matmul(wps[0:DV, :], warm[:, 0:DV], warm[:],
                                 start=True, stop=True, skip_group_check=True)
            # reader so the verifier accepts the warmup psum writes and the
            # pool ring advances
            nc.vector.tensor_copy(warm[0:1, 0:1], wps[0:1, 0:1])

        # persistent K^T and V. Each (st, head) slab is [ones(64) | V(64)]:
        # the leading ones replicate the softmax denominator onto psum
        # partitions 0-63 (O^T lands on 64-127), so normalization needs no
        # partition moves. The big ones memset runs on gpsimd, off the
        # DVE/critical path, while startup DMAs are still in flight.
        kT_sb = kvp.tile([P, NKC, L], bf16, tag="kT")
        vall = kvp.tile([P, NST * HL * P], bf16, tag="v")
        vv_c = vall[:].rearrange("p (n c) -> p n c", c=P)
        nc.gpsimd.memset(
            vall[:].rearrange("p (n t c) -> p n t c", t=2, c=DV)[:, :, 0, :], 1.0)

        W = {"q": (wq_sb, bq_sb), "k": (wk_sb, bk_sb)}

        def load_x(lc):
            xs = []
            for nm, dram, q in (("xq", xqT, nc.scalar), ("xk", xkT, nc.gpsimd),
                                ("xv", xvT, nc.gpsimd)):
                t = xp.tile([P, NDC, LC], bf16, tag=nm, name=nm)
                q.dma_start(t[:], dram[lc])
                xs.append(t)
            return xs

        def evict_qk(which, kc, ps, qt_t, lc):
            b_sb = W[which][1]
            dst = (qt_t[:, kc, :] if which == "q"
                   else kT_sb[:, kc, bass.ts(lc, LC)])
            nc.vector.tensor_scalar_add(dst, ps[:], b_sb[:, kc:kc + 1])

        def evict_v(st, ps):
            vw = (vall[:].rearrange("p (n t c) -> p n t c", t=2, c=DV)
                  [:, st * HL:(st + 1) * HL, 1, :])
            nc.vector.tensor_tensor(
                vw, ps[:].rearrange("p (h c) -> p h c", c=DV),
                bv_sb[:].rearrange("p (h c) -> p h c", c=DV),
                mybir.AluOpType.add)

        def proj_groups(lc, xs, qt_t):
            """Filler granularity: one closure per 8-matmul group."""
            xq_t, xk_t, xv_t = xs
            groups = []

            def qk_group(which, kc):
                def emit():
                    w_sb = W[which][0]
                    x_t = xq_t if which == "q" else xk_t
                    ps = ps_pj.tile([P, LC], f32, tag="ps_pj", name="ps_pj")
                    for dc in range(NDC):
                        nc.tensor.matmul(ps[:], w_sb[:, kc, dc, :],
                                         x_t[:, dc, :],
                                         start=(dc == 0), stop=(dc == NDC - 1))
                    evict_qk(which, kc, ps, qt_t, lc)
                return emit

            def v_group(j):
                def emit():
                    st = lc * (LC // P) + j
                    ps = ps_pj.tile([P, LC], f32, tag="ps_pj", name="ps_pj")
                    for dc in range(NDC):
                        nc.tensor.matmul(ps[:], xv_t[:, dc, bass.ts(j, P)],
                                         wv_sb[:, dc, :],
                                         start=(dc == 0), stop=(dc == NDC - 1))
                    evict_v(st, ps)
                return emit

            for kc in range(NKC):
                groups.append(qk_group("q", kc))
            for kc in range(NKC):
                groups.append(qk_group("k", kc))
            for j in range(LC // P):
                groups.append(v_group(j))
            return groups

        def outproj_groups(lc, at_t):
            groups = []

            def op_group(lt):
                def emit():
                    o_sb = osb.tile([P, D], f32, tag="o_sb", name="o_sb")
                    for n in range(2):
                        ps = ps_pj.tile([P, LC], f32, tag="ps_pj", name="ps_pj")
                        for hc in range(NKC):
                            nc.tensor.matmul(ps[:], at_t[hc][:, bass.ts(lt, P)],
                                             wo_sb[:, hc, bass.ts(n, 512)],
                                             start=(hc == 0), stop=(hc == NKC - 1))
                        nc.vector.tensor_copy(o_sb[:, bass.ts(n, 512)], ps[:])
                    nc.sync.dma_start(
                        outp[lc * LC + lt * P: lc * LC + (lt + 1) * P, :], o_sb[:])
                return emit

            for lt in range(LC // P):
                groups.append(op_group(lt))
            return groups

        def attention(lc, qt_t, fillers, pops=(2, 2, 2, 2)):
            n_st = (lc + 1) * (LC // P)
            at_t = [atp.tile([P, LC], bf16, tag=f"at{hc}", name=f"at{hc}")
                    for hc in range(NKC)]
            for hp in range(NKC):
                h0, h1 = 2 * hp, 2 * hp + 1
                po0 = ps_o.tile([P, LC], f32, tag="ps_o", name="ps_o")
                po1 = ps_o.tile([P, LC], f32, tag="ps_o", name="ps_o")

                def mm1(st):
                    # S^T: two heads packed on PE row halves, one 2-bank psum
                    jj = st - lc * (LC // P)
                    nc0 = jj * P if jj >= 0 else 0
                    s01 = ps_s.tile([P, 2 * LC], f32, tag="ps_s", name="ps_s")
                    nc.tensor.matmul(s01[:, nc0:LC], kT_sb[0:64, hp, bass.ts(st, P)],
                                     qt_t[0:64, hp, nc0:], start=True, stop=True,
                                     tile_position=(0, 0))
                    nc.tensor.matmul(s01[:, LC + nc0:], kT_sb[64:128, hp, bass.ts(st, P)],
                                     qt_t[64:128, hp, nc0:], start=True, stop=True,
                                     tile_position=(64, 0))
                    return s01, nc0

                pend = mm1(0)
                for st in range(n_st):
                    s01, nc0 = pend
                    if st + 1 < n_st:
                        pend = mm1(st + 1)  # PE one step ahead of ACT
                    pt01 = ptp.tile([P, 2 * LC], bf16, tag="pt", name="pt")
                    nc.scalar.activation(
                        pt01[:].rearrange("p (t c) -> p t c", t=2)[:, :, nc0:],
                        s01[:].rearrange("p (t c) -> p t c", t=2)[:, :, nc0:],
                        AF.Exp, bias=0.0, scale=float(SCALE))
                    if st >= lc * (LC // P):
                        # causal mask: zero exp() on the diagonal 128x128
                        # block where l < s (gpsimd, off the DVE/ACT path)
                        dview = (pt01[:].rearrange("p (t c) -> p t c", t=2)
                                 [:, :, nc0:nc0 + P])
                        nc.gpsimd.affine_select(
                            out=dview, in_=dview,
                            compare_op=mybir.AluOpType.is_ge,
                            fill=0.0, base=0,
                            pattern=[[0, 2], [1, P]], channel_multiplier=-1,
                        )
                    nc.tensor.matmul(po0[:, nc0:], vv_c[:, st * HL + h0, :],
                                     pt01[:, nc0:LC],
                                     start=(st == 0), stop=(st == n_st - 1))
                    nc.tensor.matmul(po1[:, nc0:], vv_c[:, st * HL + h1, :],
                                     pt01[:, LC + nc0:],
                                     start=(st == 0), stop=(st == n_st - 1))

                # PE filler(s) emitted first: they are independent of the
                # normalize chain and keep PE fed while it runs
                for _ in range(pops[hp]):
                    if fillers:
                        fillers.pop(0)()

                # normalize straight out of PSUM: den replicated on
                # partitions 0-63, O^T on 64-127
                rb0 = nrm.tile([64, LC], f32, tag="rb", name="rb")
                nc.vector.reciprocal_approx_fast(rb0[:], po0[0:64, :])
                nc.vector.tensor_mul(at_t[hp][0:64, :], po0[64:128, :], rb0[:])
                rb1 = nrm.tile([64, LC], f32, tag="rb", name="rb")
                nc.vector.reciprocal_approx_fast(rb1[:], po1[0:64, :])
                nc.vector.tensor_mul(at_t[hp][64:128, :], po1[64:128, :], rb1[:])
            return at_t

        # ---- pipeline ----
        # proj(0) runs alone, with warmup batches between groups to soak up
        # DMA-wait gaps; proj(lc+1)/outproj(lc-1) groups fill
        # normalize-chain gaps in attention(lc). proj(3) is split: its
        # kc=0,1 q/k groups + v groups run inside attention(2); its kc=2,3
        # q/k groups run inside attention(3) just before the head pairs
        # that need them.
        qt0 = qp.tile([P, NKC, LC], bf16, tag="qt", name="qt")
        pg0 = proj_groups(0, x0, qt0)
        warmup(16)
        for gi, g in enumerate(pg0):
            g()
            if gi in (0, 1, 7):
                warmup(4)
        nc.scalar.dma_start(wo_sb[:], wo[:])
        qt_cur = qt0
        at_prev = None
        pj3_late = []
        for lc in range(NLC):
            fillers = []
            if lc + 1 < NLC:
                xs = load_x(lc + 1)
                qt_nxt = qp.tile([P, NKC, LC], bf16, tag="qt", name="qt")
                pg = proj_groups(lc + 1, xs, qt_nxt)
                if lc + 1 == NLC - 1:
                    # q0,k0,q1,k1 + all v groups now; q2,k2,q3,k3 deferred
                    fillers += [pg[0], pg[4], pg[1], pg[5]] + pg[8:12]
                    pj3_late = [pg[2], pg[6], pg[3], pg[7]]
                else:
                    fillers += pg
            else:
                fillers += pj3_late
            if at_prev is not None:
                fillers += outproj_groups(lc - 1, at_prev)
            at_t = attention(lc, qt_cur, fillers,
                             pops=(2, 2, 2, 1) if lc == NLC - 1 else (2, 2, 2, 2))
            if lc == NLC - 1:
                # independent PE work covering the last pair's normalize
                # chain so HAM stays warm into the final out-projection
                warmup(8)
            for g in fillers:
                g()
            fillers.clear()
            if lc + 1 < NLC:
                qt_cur = qt_nxt
            at_prev = at_t
        for g in outproj_groups(NLC - 1, at_prev):
            g()

    nc.compile()
    return nc


def _get_compiled():
    global _COMPILED
    if _COMPILED is None:
        _COMPILED = _build()
    return _COMPILED


def kernel(queries, keys, values, Wq, bq, Wk, bk, Wv, bv, Wo, bo):
    global LAST_RESULTS
    nc = _get_compiled()

    queries = np.asarray(queries, np.float32)
    keys = np.asarray(keys, np.float32)
    values = np.asarray(values, np.float32)

    def pack_x(x):
        # (L, D) -> (NLC, P, NDC, LC): [lc, p, dc, l] = x[lc*LC+l, dc*P+p]
        t = x.T.reshape(NDC, P, NLC, LC)          # [dc, p, lc, l]
        return np.ascontiguousarray(t.transpose(2, 1, 0, 3)).astype(BF16)

    xT = {}
    for b in range(B):
        xT[("q", b)] = pack_x(np.asarray(queries[b]))
        xT[("k", b)] = pack_x(np.asarray(keys[b]))
        xT[("v", b)] = pack_x(np.asarray(values[b]))

    wslice = {}
    for g in range(2):
        sl = slice(DH * g, DH * (g + 1))

        def pack_w_kc(w):
            # (D, DH-slice) -> (NKC, P, NDC, P) kc-major
            return np.ascontiguousarray(
                w.reshape(NDC, P, NKC, P).transpose(2, 1, 0, 3)).astype(BF16)

        wslice[("wq", g)] = pack_w_kc(np.asarray(Wq, np.float32)[:, sl])
        wslice[("wk", g)] = pack_w_kc(np.asarray(Wk, np.float32)[:, sl])
        wslice[("wv", g)] = np.ascontiguousarray(
            np.asarray(Wv, np.float32)[:, sl]
            .reshape(NDC, P, DH).transpose(1, 0, 2)).astype(BF16)
        wslice[("wo", g)] = np.ascontiguousarray(
            np.asarray(Wo, np.float32)[sl, :].reshape(NKC, P, D).transpose(1, 0, 2)
        ).astype(BF16)
        wslice[("bq", g)] = np.ascontiguousarray(
            np.asarray(bq, np.float32)[sl].reshape(NKC, P).T)
        wslice[("bk", g)] = np.ascontiguousarray(
            np.asarray(bk, np.float32)[sl].reshape(NKC, P).T)
        wslice[("bv", g)] = np.ascontiguousarray(
            np.broadcast_to(np.asarray(bv, np.float32)[sl], (P, DH)))

    in_maps = []
    for c in range(N_CORES):
        b, g = c // 2, c % 2
        in_maps.append({
            "xqT": xT[("q", b)], "xkT": xT[("k", b)], "xvT": xT[("v", b)],
            "wq": wslice[("wq", g)], "wk": wslice[("wk", g)],
            "wv": wslice[("wv", g)], "wo": wslice[("wo", g)],
            "bq": wslice[("bq", g)], "bk": wslice[("bk", g)],
            "bv": wslice[("bv", g)],
        })

    res = run_bass_kernel_spmd(nc, in_maps, list(range(N_CORES)), trace=TRACE)
    LAST_RESULTS = res

    bo32 = np.asarray(bo, np.float32)
    out = np.empty((B, L, D), np.float32)
    for b in range(B):
        out[b] = res.results[2 * b]["outp"] + res.results[2 * b + 1]["outp"] + bo32
    return out


# revision 33
# speedup vs baseline: 1.0400x; 1.0253x over previous
"""Causal multi-head attention layer on 8 trn2 NeuronCores.

Sharding: 8 cores = 4 batches x 2 head-groups. Core c handles batch c//2 and
heads [8*(c%2), 8*(c%2)+8). Each core runs QKV projections for its 512-wide
head slice, causal flash attention for 8 heads, and a partial output
projection (its 512 rows of Wo). Host sums the two partials per batch + bo.

Problem constants (hardcoded per contract): B=4, L=2048, D=1024, H=16, DK=DV=64.
"""

import sys

import os
for _p in ("/opt/trn_rl_repo", "/root/.axon_site/_ro/trn_rl_repo"):
    if os.path.isdir(_p) and _p not in sys.path:
        sys.path.insert(0, _p)

import numpy as np
import ml_dtypes

import concourse.bass as bass
import concourse.tile as tile
from concourse import bacc, mybir
from concourse.bass_utils import run_bass_kernel_spmd
BF16 = ml_dtypes.bfloat16

B, L, D, H, DK, DV = 4, 2048, 1024, 16, 64, 64
N_CORES = 8
HL = 8          # heads per core
DH = 512        # local head dim (HL * DK)
P = 128
LC = 512        # l-chunk
NLC = L // LC   # 4
NDC = D // P    # 8 contraction chunks for projections
NKC = DH // P   # 4 dk chunks
NST = L // P    # 16 s tiles
SCALE = 1.0 / np.sqrt(DK)

TRACE = False          # set by test harness for profiling runs
LAST_RESULTS = None    # BassKernelResults of the last run (for profiling)

_COMPILED = None


def _build():
    f32 = mybir.dt.float32
    bf16 = mybir.dt.bfloat16
    AF = mybir.ActivationFunctionType

    nc = bacc.Bacc("TRN2", target_bir_lowering=False, debug=False,
                   num_devices=N_CORES)

    xqT = nc.dram_tensor("xqT", [NLC, P, NDC, LC], bf16, kind="ExternalInput").ap()
    xkT = nc.dram_tensor("xkT", [NLC, P, NDC, LC], bf16, kind="ExternalInput").ap()
    xvT = nc.dram_tensor("xvT", [NLC, P, NDC, LC], bf16, kind="ExternalInput").ap()
    # wq/wk are kc-major so per-kc weight slices are contiguous DMAs
    wq = nc.dram_tensor("wq", [NKC, P, NDC, P], bf16, kind="ExternalInput").ap()
    wk = nc.dram_tensor("wk", [NKC, P, NDC, P], bf16, kind="ExternalInput").ap()
    wv = nc.dram_tensor("wv", [P, NDC, DH], bf16, kind="ExternalInput").ap()
    wo = nc.dram_tensor("wo", [P, NKC, D], bf16, kind="ExternalInput").ap()
    bq = nc.dram_tensor("bq", [P, NKC], f32, kind="ExternalInput").ap()
    bk = nc.dram_tensor("bk", [P, NKC], f32, kind="ExternalInput").ap()
    bv = nc.dram_tensor("bv", [P, DH], f32, kind="ExternalInput").ap()
    outp = nc.dram_tensor("outp", [L, D], f32, kind="ExternalOutput").ap()

    from contextlib import ExitStack

    with tile.TileContext(nc) as tc, ExitStack() as ctx:
        const = ctx.enter_context(tc.tile_pool(name="const", bufs=1))
        kvp = ctx.enter_context(tc.tile_pool(name="kv", bufs=1))
        xp = ctx.enter_context(tc.tile_pool(name="x", bufs=2))
        qp = ctx.enter_context(tc.tile_pool(name="qt", bufs=2))
        ptp = ctx.enter_context(tc.tile_pool(name="pt", bufs=6))
        atp = ctx.enter_context(tc.tile_pool(name="at", bufs=2))
        osb = ctx.enter_context(tc.tile_pool(name="osb", bufs=3))
        nrm = ctx.enter_context(tc.tile_pool(name="nrm", bufs=3))
        ps_pj = ctx.enter_context(tc.tile_pool(name="ps_pj", bufs=2, space="PSUM"))
        ps_s = ctx.enter_context(tc.tile_pool(name="ps_s", bufs=2, space="PSUM"))
        ps_o = ctx.enter_context(tc.tile_pool(name="ps_o", bufs=2, space="PSUM"))

        # ---- initial loads: 3 DMA queues, need-ordered. Startup is
        # HBM-bound (~6MB before attention(0)), so critical bytes go first.
        wq_sb = const.tile([P, NKC, NDC, P], bf16, tag="wq")
        wk_sb = const.tile([P, NKC, NDC, P], bf16, tag="wk")
        wv_sb = const.tile([P, NDC, DH], bf16, tag="wv")
        x0 = [xp.tile([P, NDC, LC], bf16, tag=nm, name=nm)
              for nm in ("xq", "xk", "xv")]
        # sync: wq kc0 -> xq half -> wq rest -> xk half -> wv
        nc.sync.dma_start(wq_sb[:, 0], wq[0])
        nc.sync.dma_start(x0[0][:, 0:4], xqT[0][:, 0:4])
        for kc in range(1, NKC):
            nc.sync.dma_start(wq_sb[:, kc], wq[kc])
        nc.sync.dma_start(x0[1][:, 0:4], xkT[0][:, 0:4])
        nc.sync.dma_start(wv_sb[:], wv[:])
        # scalar: xq other half -> wk 0,1 -> xk other half -> biases
        nc.scalar.dma_start(x0[0][:, 4:8], xqT[0][:, 4:8])
        nc.scalar.dma_start(wk_sb[:, 0], wk[0])
        nc.scalar.dma_start(wk_sb[:, 1], wk[1])
        nc.scalar.dma_start(x0[1][:, 4:8], xkT[0][:, 4:8])
        bq_sb = const.tile([P, NKC], f32, tag="bq")
        nc.scalar.dma_start(bq_sb[:], bq[:])
        bk_sb = const.tile([P, NKC], f32, tag="bk")
        nc.scalar.dma_start(bk_sb[:], bk[:])
        bv_sb = const.tile([P, DH], f32, tag="bv")
        nc.scalar.dma_start(bv_sb[:], bv[:])
        wo_sb = const.tile([P, NKC, D], bf16, tag="wo")
        # gpsimd: wk 2,3 -> xv (wo is deferred until the startup crunch ends)
        nc.gpsimd.dma_start(wk_sb[:, 2], wk[2])
        nc.gpsimd.dma_start(wk_sb[:, 3], wk[3])
        nc.gpsimd.dma_start(x0[2][:], xvT[0])

        # ---- PE warmup: batches of matmuls with no data deps, interleaved
        # between the first projection groups. They soak up DMA-wait gaps so
        # HAM reaches (and keeps) K=8/8 through the byte-bound start phase.
        warm = const.tile([P, LC], bf16, tag="warm")
        nc.vector.memset(warm[:], 1.0)

        def warmup(n):
            wps = ps_pj.tile([P, LC], f32, tag="ps_pj", name="warm")
            for _ in range(n):
                nc.tensor.matmul(wps[0:DV, :], warm[:, 0:DV], warm[:],
                                 start=True, stop=True, skip_group_check=True)
            # reader so the verifier accepts the warmup psum writes and the
            # pool ring advances
            nc.vector.tensor_copy(warm[0:1, 0:1], wps[0:1, 0:1])

        # persistent K^T and V. Each (st, head) slab is [ones(64) | V(64)]:
        # the leading ones replicate the softmax denominator onto psum
        # partitions 0-63 (O^T lands on 64-127), so normalization needs no
        # partition moves. The big ones memset runs on gpsimd, off the
        # DVE/critical path, while startup DMAs are still in flight.
        kT_sb = kvp.tile([P, NKC, L], bf16, tag="kT")
        vall = kvp.tile([P, NST * HL * P], bf16, tag="v")
        vv_c = vall[:].rearrange("p (n c) -> p n c", c=P)
        nc.gpsimd.memset(
            vall[:].rearrange("p (n t c) -> p n t c", t=2, c=DV)[:, :, 0, :], 1.0)

        W = {"q": (wq_sb, bq_sb), "k": (wk_sb, bk_sb)}

        def load_x(lc):
            xs = []
            for nm, dram, q in (("xq", xqT, nc.scalar), ("xk", xkT, nc.gpsimd),
                                ("xv", xvT, nc.gpsimd)):
                t = xp.tile([P, NDC, LC], bf16, tag=nm, name=nm)
                q.dma_start(t[:], dram[lc])
                xs.append(t)
            return xs

        def evict_qk(which, kc, ps, qt_t, lc):
            b_sb = W[which][1]
            dst = (qt_t[:, kc, :] if which == "q"
                   else kT_sb[:, kc, bass.ts(lc, LC)])
            nc.vector.tensor_scalar_add(dst, ps[:], b_sb[:, kc:kc + 1])

        def evict_v(st, ps):
            vw = (vall[:].rearrange("p (n t c) -> p n t c", t=2, c=DV)
                  [:, st * HL:(st + 1) * HL, 1, :])
            nc.vector.tensor_tensor(
                vw, ps[:].rearrange("p (h c) -> p h c", c=DV),
                bv_sb[:].rearrange("p (h c) -> p h c", c=DV),
                mybir.AluOpType.add)

        def proj_groups(lc, xs, qt_t):
            """Filler granularity: one closure per 8-matmul group."""
            xq_t, xk_t, xv_t = xs
            groups = []

            def qk_group(which, kc):
                def emit():
                    w_sb = W[which][0]
                    x_t = xq_t if which == "q" else xk_t
                    ps = ps_pj.tile([P, LC], f32, tag="ps_pj", name="ps_pj")
                    for dc in range(NDC):
                        nc.tensor.matmul(ps[:], w_sb[:, kc, dc, :],
                                         x_t[:, dc, :],
                                         start=(dc == 0), stop=(dc == NDC - 1))
                    evict_qk(which, kc, ps, qt_t, lc)
                return emit

            def v_group(j):
                def emit():
                    st = lc * (LC // P) + j
                    ps = ps_pj.tile([P, LC], f32, tag="ps_pj", name="ps_pj")
                    for dc in range(NDC):
                        nc.tensor.matmul(ps[:], xv_t[:, dc, bass.ts(j, P)],
                                         wv_sb[:, dc, :],
                                         start=(dc == 0), stop=(dc == NDC - 1))
                    evict_v(st, ps)
                return emit

            for kc in range(NKC):
                groups.append(qk_group("q", kc))
            for kc in range(NKC):
                groups.append(qk_group("k", kc))
            for j in range(LC // P):
                groups.append(v_group(j))
            return groups

        def outproj_groups(lc, at_t):
            groups = []

            def op_group(lt):
                def emit():
                    o_sb = osb.tile([P, D], f32, tag="o_sb", name="o_sb")
                    for n in range(2):
                        ps = ps_pj.tile([P, LC], f32, tag="ps_pj", name="ps_pj")
                        for hc in range(NKC):
                            nc.tensor.matmul(ps[:], at_t[hc][:, bass.ts(lt, P)],
                                             wo_sb[:, hc, bass.ts(n, 512)],
                                             start=(hc == 0), stop=(hc == NKC - 1))
                        nc.vector.tensor_copy(o_sb[:, bass.ts(n, 512)], ps[:])
                    nc.sync.dma_start(
                        outp[lc * LC + lt * P: lc * LC + (lt + 1) * P, :], o_sb[:])
                return emit

            for lt in range(LC // P):
                groups.append(op_group(lt))
            return groups

        def attention(lc, qt_t, fillers, pops=(2, 2, 2, 2)):
            n_st = (lc + 1) * (LC // P)
            at_t = [atp.tile([P, LC], bf16, tag=f"at{hc}", name=f"at{hc}")
                    for hc in range(NKC)]
            for hp in range(NKC):
                h0, h1 = 2 * hp, 2 * hp + 1
                po0 = ps_o.tile([P, LC], f32, tag="ps_o", name="ps_o")
                po1 = ps_o.tile([P, LC], f32, tag="ps_o", name="ps_o")

                def mm1(st):
                    # S^T: two heads packed on PE row halves, one 2-bank psum
                    jj = st - lc * (LC // P)
                    nc0 = jj * P if jj >= 0 else 0
                    s01 = ps_s.tile([P, 2 * LC], f32, tag="ps_s", name="ps_s")
                    nc.tensor.matmul(s01[:, nc0:LC], kT_sb[0:64, hp, bass.ts(st, P)],
                                     qt_t[0:64, hp, nc0:], start=True, stop=True,
                                     tile_position=(0, 0))
                    nc.tensor.matmul(s01[:, LC + nc0:], kT_sb[64:128, hp, bass.ts(st, P)],
                                     qt_t[64:128, hp, nc0:], start=True, stop=True,
                                     tile_position=(64, 0))
                    return s01, nc0

                pend = mm1(0)
                for st in range(n_st):
                    s01, nc0 = pend
                    if st + 1 < n_st:
                        pend = mm1(st + 1)  # PE one step ahead of ACT
                    pt01 = ptp.tile([P, 2 * LC], bf16, tag="pt", name="pt")
                    nc.scalar.activation(
                        pt01[:].rearrange("p (t c) -> p t c", t=2)[:, :, nc0:],
                        s01[:].rearrange("p (t c) -> p t c", t=2)[:, :, nc0:],
                        AF.Exp, bias=0.0, scale=float(SCALE))
                    if st >= lc * (LC // P):
                        # causal mask: zero exp() on the diagonal 128x128
                        # block where l < s (gpsimd, off the DVE/ACT path)
                        dview = (pt01[:].rearrange("p (t c) -> p t c", t=2)
                                 [:, :, nc0:nc0 + P])
                        nc.gpsimd.affine_select(
                            out=dview, in_=dview,
                            compare_op=mybir.AluOpType.is_ge,
                            fill=0.0, base=0,
                            pattern=[[0, 2], [1, P]], channel_multiplier=-1,
                        )
                    nc.tensor.matmul(po0[:, nc0:], vv_c[:, st * HL + h0, :],
                                     pt01[:, nc0:LC],
                                     start=(st == 0), stop=(st == n_st - 1))
                    nc.tensor.matmul(po1[:, nc0:], vv_c[:, st * HL + h1, :],
                                     pt01[:, LC + nc0:],
                                     start=(st == 0), stop=(st == n_st - 1))

                # PE filler(s) emitted first: they are independent of the
                # normalize chain and keep PE fed while it runs
                for _ in range(pops[hp]):
                    if fillers:
                        fillers.pop(0)()

                # normalize straight out of PSUM: den replicated on
                # partitions 0-63, O^T on 64-127
                rb0 = nrm.tile([64, LC], f32, tag="rb", name="rb")
                nc.vector.reciprocal_approx_fast(rb0[:], po0[0:64, :])
                nc.vector.tensor_mul(at_t[hp][0:64, :], po0[64:128, :], rb0[:])
                rb1 = nrm.tile([64, LC], f32, tag="rb", name="rb")
                nc.vector.reciprocal_approx_fast(rb1[:], po1[0:64, :])
                nc.vector.tensor_mul(at_t[hp][64:128, :], po1[64:128, :], rb1[:])
            return at_t

        # ---- pipeline ----
        # proj(0) runs alone, with warmup batches between groups to soak up
        # DMA-wait gaps; proj(lc+1)/outproj(lc-1) groups fill
        # normalize-chain gaps in attention(lc). proj(3) is split: its
        # kc=0,1 q/k groups + v groups run inside attention(2); its kc=2,3
        # q/k groups run inside attention(3) just before the head pairs
        # that need them.
        qt0 = qp.tile([P, NKC, LC], bf16, tag="qt", name="qt")
        pg0 = proj_groups(0, x0, qt0)
        for g in pg0:
            g()
        nc.scalar.dma_start(wo_sb[:], wo[:])
        qt_cur = qt0
        at_prev = None
        pj3_late = []
        for lc in range(NLC):
            fillers = []
            if lc + 1 < NLC:
                xs = load_x(lc + 1)
                qt_nxt = qp.tile([P, NKC, LC], bf16, tag="qt", name="qt")
                pg = proj_groups(lc + 1, xs, qt_nxt)
                if lc + 1 == NLC - 1:
                    # q0,k0,q1,k1 + all v groups now; q2,k2,q3,k3 deferred
                    fillers += [pg[0], pg[4], pg[1], pg[5]] + pg[8:12]
                    pj3_late = [pg[2], pg[6], pg[3], pg[7]]
                else:
                    fillers += pg
            else:
                fillers += pj3_late
            if at_prev is not None:
                fillers += outproj_groups(lc - 1, at_prev)
            at_t = attention(lc, qt_cur, fillers,
                             pops=(2, 2, 2, 1) if lc == NLC - 1 else (2, 2, 2, 2))
            if lc == NLC - 1:
                # independent PE work covering the last pair's normalize
                # chain so HAM stays warm into the final out-projection
                warmup(8)
            for g in fillers:
                g()
            fillers.clear()
            if lc + 1 < NLC:
                qt_cur = qt_nxt
            at_prev = at_t
        for g in outproj_groups(NLC - 1, at_prev):
            g()

    nc.compile()
    return nc


def _get_compiled():
    global _COMPILED
    if _COMPILED is None:
        _COMPILED = _build()
    return _COMPILED


def kernel(queries, keys, values, Wq, bq, Wk, bk, Wv, bv, Wo, bo):
    global LAST_RESULTS
    nc = _get_compiled()

    queries = np.asarray(queries, np.float32)
    keys = np.asarray(keys, np.float32)
    values = np.asarray(values, np.float32)

    def pack_x(x):
        # (L, D) -> (NLC, P, NDC, LC): [lc, p, dc, l] = x[lc*LC+l, dc*P+p]
        t = x.T.reshape(NDC, P, NLC, LC)          # [dc, p, lc, l]
        return np.ascontiguousarray(t.transpose(2, 1, 0, 3)).astype(BF16)

    xT = {}
    for b in range(B):
        xT[("q", b)] = pack_x(np.asarray(queries[b]))
        xT[("k", b)] = pack_x(np.asarray(keys[b]))
        xT[("v", b)] = pack_x(np.asarray(values[b]))

    wslice = {}
    for g in range(2):
        sl = slice(DH * g, DH * (g + 1))

        def pack_w_kc(w):
            # (D, DH-slice) -> (NKC, P, NDC, P) kc-major
            return np.ascontiguousarray(
                w.reshape(NDC, P, NKC, P).transpose(2, 1, 0, 3)).astype(BF16)

        wslice[("wq", g)] = pack_w_kc(np.asarray(Wq, np.float32)[:, sl])
        wslice[("wk", g)] = pack_w_kc(np.asarray(Wk, np.float32)[:, sl])
        wslice[("wv", g)] = np.ascontiguousarray(
            np.asarray(Wv, np.float32)[:, sl]
            .reshape(NDC, P, DH).transpose(1, 0, 2)).astype(BF16)
        wslice[("wo", g)] = np.ascontiguousarray(
            np.asarray(Wo, np.float32)[sl, :].reshape(NKC, P, D).transpose(1, 0, 2)
        ).astype(BF16)
        wslice[("bq", g)] = np.ascontiguousarray(
            np.asarray(bq, np.float32)[sl].reshape(NKC, P).T)
        wslice[("bk", g)] = np.ascontiguousarray(
            np.asarray(bk, np.float32)[sl].reshape(NKC, P).T)
        wslice[("bv", g)] = np.ascontiguousarray(
            np.broadcast_to(np.asarray(bv, np.float32)[sl], (P, DH)))

    in_maps = []
    for c in range(N_CORES):
        b, g = c // 2, c % 2
        in_maps.append({
            "xqT": xT[("q", b)], "xkT": xT[("k", b)], "xvT": xT[("v", b)],
            "wq": wslice[("wq", g)], "wk": wslice[("wk", g)],
            "wv": wslice[("wv", g)], "wo": wslice[("wo", g)],
            "bq": wslice[("bq", g)], "bk": wslice[("bk", g)],
            "bv": wslice[("bv", g)],
        })

    res = run_bass_kernel_spmd(nc, in_maps, list(range(N_CORES)), trace=TRACE)
    LAST_RESULTS = res

    bo32 = np.asarray(bo, np.float32)
    out = np.empty((B, L, D), np.float32)
    for b in range(B):
        out[b] = res.results[2 * b]["outp"] + res.results[2 * b + 1]["outp"] + bo32
    return out
